# revision 3
# baseline (speedup 1.0000x reference)
"""Transformer-XL rel-pos MHA on 8 trn2 cores — v3.

v3 changes vs v2:
- DoubleRow fp8 matmuls (0.5 cyc/col) for all K=1024 projections (kt/v/q/p),
  the out-projection, and the AV accumulation (t-tile pairs).
- exp() writes attention weights as fp8e4 into paired [128,2048] tiles; the
  val tile (vsb) and attention output (awvt/a2a/awvf) are fp8 too.
- The AllToAll is split into two half-size collectives (per-hl), which
  empirically removes a ~200us/iter HW penalty the single 1MB A2A pays.


Sharding: tensor-parallel over heads (2/core), AllToAll, token-sharded
out-proj+LN (as v1). Key changes vs v1:

- R (position scores, (i, r) coords) is written to DRAM with row pitch 2049
  (one zero element appended per 2048-wide row). Then the Transformer-XL
  rel-shift read is a SINGLE clean strided transpose-DMA per (hl, t):
      shifted[i, j] = W[i*2048 + 1023 + j]
  (the j-i==1025 zero and the j-i>=1026 wrap fall out automatically), killing
  the d2 reads, affine_select masks and mask adds of v1.
- Attention runs per (b, hl) over the full 1024-query range: one [128,1024]
  PSUM score tile per t (2 banks), one [128,1024] transpose read, one
  [128,1024] exp. Shifted-score add alternates PE ident-matmul / DVE add.
- R matmuls issue as (hl0, hl1) adjacent pairs -> tile_position row packing.
- R-chunk production for batch b+1 is interleaved into attn(b)'s t-loop so
  PE never stalls on the copy-paced R pipeline.
- PSUM: scores 2x[128,1024] + AV 2x[65,512] + R 2x[128,512] = 8 banks.
"""
import numpy as np
import ml_dtypes

import concourse.bass as bass
import concourse.mybir as mybir
import concourse.tile as tile
from concourse import bacc
from concourse.bass_utils import run_bass_kernel_spmd
from concourse.masks import make_identity
import bass_rust

BF = mybir.dt.bfloat16
F32 = mybir.dt.float32
F8 = mybir.dt.float8e4
WSC = 16.0
AF = mybir.ActivationFunctionType
DR = mybir.MatmulPerfMode.DoubleRow
ALU = mybir.AluOpType
bf16 = ml_dtypes.bfloat16

S = 1024
PREV = 1024
T = 2048
B = 4
D = 1024
H = 16
d = 64
NC = 8
SCALE = 1.0 / 8.0
LN_EPS = 1e-5
PITCH = T + 1              # 2049: W row pitch (includes the zero gap)
SZH = S * PITCH            # per-(hl) W region, elements


def _ap(handle, offset, pattern):
    return bass_rust.AP(tensor=handle, offset=offset, ap=pattern)


def _proj_b(nc, tc, io, pools, b):
    """kt / v / q(+u,+v bias) projections for batch b (nt tiles 4b..4b+3)."""
    res = pools["res"]
    xtp, ps1, psv = pools["xtp"], pools["ps1"], pools["psv"]
    kt, vsb, qu, qv = res["kt"], res["vsb"], res["qu"], res["qv"]
    wk, wv, wq = res["wk"], res["wv"], res["wq"]
    ub, vb = res["ub"], res["vb"]
    for nt in range(4 * b, 4 * b + 4):
        xtile = xtp.tile([128, 8 * 512], F8, tag="xt", name="xt")
        xeng = nc.sync if nt % 2 == 0 else nc.scalar
        xeng.dma_start(
            xtile[:],
            _ap(io["xt"], nt * 512,
                [[B * T, 128], [B * T * 128, 8], [1, 512]]))
        x3 = xtile[:].rearrange("p (kd n) -> p kd n", kd=8)
        wk3 = wk[:].rearrange("p (kd n) -> p kd n", kd=8)
        wv3 = wv[:].rearrange("p (kd n) -> p kd n", kd=8)
        wq3 = wq[:].rearrange("p (kd n) -> p kd n", kd=8)
        ps = ps1.tile([128, 512], F32, tag="mm", name="mm")
        for k2 in range(4):
            nc.tensor.matmul(
                ps[:], lhsT=wk3[:, 2 * k2:2 * k2 + 2, :],
                rhs=x3[:, 2 * k2:2 * k2 + 2, :],
                start=(k2 == 0), stop=(k2 == 3), perf_mode=DR)
        nc.scalar.activation(kt[:, nt * 512:(nt + 1) * 512], ps[:], AF.Identity,
                             scale=1.0 / WSC)
        for sub in range(4):
            pv = psv.tile([128, 128], F32, tag="v", name="v")
            for kd in range(8):
                nc.tensor.matmul(
                    pv[:],
                    lhsT=xtile[:, kd * 512 + sub * 128: kd * 512 + (sub + 1) * 128],
                    rhs=wv[:, kd * 128:(kd + 1) * 128],
                    start=(kd == 0), stop=(kd == 7))
            g = nt * 4 + sub
            nc.scalar.activation(
                vsb[:, g * 160: g * 160 + 64], pv[:, 0:64], AF.Identity,
                scale=1.0 / WSC)
            nc.scalar.activation(
                vsb[:, g * 160 + 80: g * 160 + 144], pv[:, 64:128],
                AF.Identity, scale=1.0 / WSC)
        if nt % 4 >= 2:
            pq = ps1.tile([128, 512], F32, tag="mm", name="mm")
            for k2 in range(4):
                nc.tensor.matmul(
                    pq[:], lhsT=wq3[:, 2 * k2:2 * k2 + 2, :],
                    rhs=x3[:, 2 * k2:2 * k2 + 2, :],
                    start=(k2 == 0), stop=(k2 == 3), perf_mode=DR)
            qc = b * S + (nt % 4 - 2) * 512
            nc.scalar.activation(qu[:, qc:qc + 512], pq[:], AF.Identity,
                                 bias=ub[:], scale=1.0 / WSC)
            nc.scalar.activation(qv[:, qc:qc + 512], pq[:], AF.Identity,
                                 bias=vb[:], scale=1.0 / WSC)


def _r_chunks0(nc, tc, io, pools, its):
    """R for b=0, emitted inside the projection phase with its own 4-bank
    PSUM pool: [128,1024] rt-pair matmuls, one wide copy each (DVE/Act
    alternating), interleaved between batch projections so proj matmuls
    fill the copy stalls and W(0) drains during projection."""
    res = pools["res"]
    ps2, rsp = pools["psr0"], pools["rsp"]
    qv, pt = res["qv"], res["pt"]
    w_bt = io["w"][0]
    for it in its:
        rs0 = rsp.tile([128, PITCH], BF, tag="rs0", name="rs0")
        rs1 = rsp.tile([128, PITCH], BF, tag="rs1", name="rs1")
        nc.gpsimd.memset(rs0[:, T:PITCH], 0.0)
        nc.gpsimd.memset(rs1[:, T:PITCH], 0.0)
        for hl, rs in ((0, rs0), (1, rs1)):
            hs = slice(hl * 64, (hl + 1) * 64)
            for half in range(2):
                pr = ps2.tile([128, 1024], F32, tag="sc", name="sc")
                for rt2 in range(2):
                    rt = half * 2 + rt2
                    nc.tensor.matmul(
                        pr[:, rt2 * 512:(rt2 + 1) * 512],
                        lhsT=qv[hs, it * 128:(it + 1) * 128],
                        rhs=pt[hs, rt * 512:(rt + 1) * 512],
                        start=True, stop=True)
                dst = rs[:, half * 1024:(half + 1) * 1024]
                if (it * 2 + half) % 2 == 0:
                    nc.vector.tensor_copy(dst, pr[:])
                else:
                    nc.scalar.activation(dst, pr[:], AF.Copy)
        for hl, rs in ((0, rs0), (1, rs1)):
            nc.sync.dma_start(
                _ap(w_bt[hl], it * 128 * PITCH,
                    [[PITCH, 128], [1, PITCH]]),
                rs[:])


def _r_chunks(nc, tc, io, pools, b, act_copy=False):
    """Emit-closures for batch b's R pipeline: 64 PSUM-tile chunks.

    Chunk (it, rt): packed MM pair (hl0 tile_pos (0,0), hl1 (64,0)) into two
    [128,512] R-pool tiles, copy to rs tiles (DVE for hl0 / Pool for hl1),
    and on rt==3 the rs -> W DMA (pitch-2049 rows, gap col pre-zeroed).
    For it<=3, rt==0 is skipped entirely (r<512 is never read when i<512):
    the copy/write start at col 512.
    """
    res = pools["res"]
    psr, rsp = pools["psr"], pools["rsp"]
    qv, pt = res["qv"], res["pt"]
    w0_t, w1_t = io["w"][b]
    chunks = []
    state = {}

    def start_it(it):
        rs = rsp.tile([128, 2 * PITCH], BF, tag="rs", name="rs")
        nc.gpsimd.memset(rs[:, T:PITCH], 0.0)
        nc.gpsimd.memset(rs[:, PITCH + T:2 * PITCH], 0.0)
        state["rs"] = rs

    def chunk(it, rt):
        def run():
            if rt == 0:
                start_it(it)
            rs = state["rs"]
            pr = psr.tile([128, 1024], F32, tag="r", name="r")
            nc.tensor.matmul(
                pr[:, 0:512],
                lhsT=qv[0:64, b * S + it * 128: b * S + (it + 1) * 128],
                rhs=pt[0:64, rt * 512:(rt + 1) * 512],
                start=True, stop=True)
            nc.tensor.matmul(
                pr[:, 512:1024],
                lhsT=qv[64:128, b * S + it * 128: b * S + (it + 1) * 128],
                rhs=pt[64:128, rt * 512:(rt + 1) * 512],
                start=True, stop=True)
            rv = rs[:].rearrange("p (k n) -> p k n", k=2)[:, :,
                                                         rt * 512:(rt + 1) * 512]
            nc.vector.tensor_copy(
                rv, pr[:].rearrange("p (k n) -> p k n", k=2))
            if rt == 3:
                nc.sync.dma_start(
                    _ap(w0_t, it * 128 * PITCH, [[PITCH, 128], [1, PITCH]]),
                    rs[:, 0:PITCH])
                nc.sync.dma_start(
                    _ap(w1_t, it * 128 * PITCH, [[PITCH, 128], [1, PITCH]]),
                    rs[:, PITCH:2 * PITCH])
        return run

    for it in range(8):
        for rt in range(4):
            chunks.append(chunk(it, rt))
    return chunks


def _attn_block(nc, tc, io, pools, b, hl, interleave):
    """Attention for (b, hl): 16 t-tiles over the full 1024-query range.

    interleave: list of closures (R chunks for b+1); 2 emitted per t-slot.
    """
    res = pools["res"]
    ps2, psav = pools["ps2"], pools["psav"]
    skp, atp, nrm = pools["skp"], pools["atp"], pools["nrm"]
    kt, qu, vsb, ident = res["kt"], res["qu"], res["vsb"], res["ident"]
    awvt = res["awvt0"] if hl == 0 else res["awvt1"]
    hs = slice(hl * 64, (hl + 1) * 64)
    w_t = io["w"][b][hl]

    pav0 = psav.tile([65, 512], F32, tag="av", name="av")
    pav1 = psav.tile([65, 512], F32, tag="av", name="av")
    pavs = (pav0, pav1)
    vs3 = vsb[:].rearrange("p (g c) -> p g c", g=64)

    def emit_av(at, tp):
        g = b * 16 + 2 * tp
        at3 = at[:].rearrange("p (k n) -> p k n", k=2)
        for i0b in range(2):
            nc.tensor.matmul(
                pavs[i0b][:],
                lhsT=vs3[:, g:g + 2, hl * 80: hl * 80 + 65],
                rhs=at3[:, :, i0b * 512:(i0b + 1) * 512],
                start=(tp == 0), stop=(tp == 7), perf_mode=DR)

    pend = None
    for tp in range(8):
        at = atp.tile([128, 2048], F8, tag="at", name="at")
        for th in range(2):
            t = 2 * tp + th
            sc = ps2.tile([128, 1024], F32, tag="sc", name="sc")
            dsk = skp.tile([128, 1024], BF, tag="d", name="d")
            nc.sync.dma_start_transpose(
                dsk[:],
                _ap(w_t, 1023 + 128 * t, [[T, 1024], [1, 128]]))
            pe_add = (hl == 0) or (t % 4 != 3)
            for i0b in range(2):
                nc.tensor.matmul(
                    sc[:, i0b * 512:(i0b + 1) * 512],
                    lhsT=kt[hs, b * T + t * 128: b * T + (t + 1) * 128],
                    rhs=qu[hs, b * S + i0b * 512: b * S + (i0b + 1) * 512],
                    start=True, stop=not pe_add)
            if pe_add:
                for i0b in range(2):
                    nc.tensor.matmul(
                        sc[:, i0b * 512:(i0b + 1) * 512], lhsT=ident[:],
                        rhs=dsk[:, i0b * 512:(i0b + 1) * 512],
                        start=False, stop=True)
            else:
                nc.vector.tensor_tensor(out=sc[:], in0=sc[:], in1=dsk[:],
                                        op=ALU.add)
            if pend is not None:
                emit_av(*pend)
                pend = None
            nc.scalar.activation(at[:, th * 1024:(th + 1) * 1024], sc[:],
                                 AF.Exp, scale=SCALE)
            if interleave:
                interleave.pop(0)()
        pend = (at, tp)
    emit_av(*pend)
    # softmax denominators + scale (row 64 of pav = sum of exp)
    for i0b in range(2):
        pav = pavs[i0b]
        rec = nrm.tile([1, 512], F32, tag="rec", name="rec")
        nc.vector.reciprocal(rec[:], pav[64:65, :])
        recb = nrm.tile([64, 512], F32, tag="recb", name="recb")
        nc.gpsimd.partition_broadcast(recb[:], rec[:])
        nc.vector.tensor_tensor(
            out=awvt[:, b * S + i0b * 512: b * S + (i0b + 1) * 512],
            in0=pav[0:64, :], in1=recb[:], op=ALU.mult)
    # stage this (b, hl) slice into the per-hl A2A input buffer
    nc.sync.dma_start(
        _ap(io["a2a_in_h"][hl], (2 * b) * 32768,
            [[512, 64], [32768, 2], [1, 512]]),
        awvt[:, b * S: b * S + 1024])


def _body(nc, tc, io, repeat=1):
    out_t = io["out"]
    with tc.tile_pool(name="res", bufs=1) as res_pool:
        res = {}
        res["kt"] = res_pool.tile([128, B * T], BF, tag="kt", name="kt")
        res["vsb"] = res_pool.tile([128, 64 * 160], F8, tag="vsb", name="vsb")
        res["qu"] = res_pool.tile([128, B * S], BF, tag="qu", name="qu")
        res["qv"] = res_pool.tile([128, B * S], BF, tag="qv", name="qv")
        res["pt"] = res_pool.tile([128, T], BF, tag="pt", name="pt")
        res["wout"] = res_pool.tile([128, 8 * D], F8, tag="wout", name="wout")
        res["wk"] = res_pool.tile([128, D], F8, tag="wk", name="wk")
        res["wv"] = res_pool.tile([128, D], F8, tag="wv", name="wv")
        res["wq"] = res_pool.tile([128, D], F8, tag="wq", name="wq")
        res["wp"] = res_pool.tile([128, D], F8, tag="wp", name="wp")
        res["awvt0"] = res_pool.tile([64, B * S], F8, tag="awvt0", name="awvt0")
        res["awvt1"] = res_pool.tile([64, B * S], F8, tag="awvt1", name="awvt1")
        res["awvf"] = res_pool.tile([128, 8 * 512], F8, tag="awvf", name="awvf")
        res["ident"] = res_pool.tile([128, 128], BF, tag="ident", name="ident")
        res["ub"] = res_pool.tile([128, 1], F32, tag="ub", name="ub")
        res["vb"] = res_pool.tile([128, 1], F32, tag="vb", name="vb")
        res["lng_b"] = res_pool.tile([128, D], F32, tag="lngb", name="lngb")
        res["lnb_b"] = res_pool.tile([128, D], F32, tag="lnbb", name="lnbb")
        res["resid"] = [res_pool.tile([128, D], F32, tag=f"resid{i}", name=f"resid{i}")
                        for i in range(4)]
        lng_r = res_pool.tile([1, D], F32, tag="lngr", name="lngr")
        lnb_r = res_pool.tile([1, D], F32, tag="lnbr", name="lnbr")

        for wt_sb, wt_h in ((res["wp"], io["wp"]), (res["wk"], io["wk"]),
                            (res["wv"], io["wv"]), (res["wq"], io["wq"])):
            nc.sync.dma_start(wt_sb[:], _ap(wt_h, 0, [[1024, 128], [1, 1024]]))
        nc.sync.dma_start(res["ub"][:], io["ub"][:])
        nc.sync.dma_start(res["vb"][:], io["vb"][:])
        nc.gpsimd.dma_start(res["wout"][:], _ap(io["wout"], 0, [[1024, 128], [131072, 8], [1, 1024]]))
        nc.gpsimd.dma_start(lng_r[:], io["lng"][:])
        nc.gpsimd.dma_start(lnb_r[:], io["lnb"][:])
        for i in range(4):
            nc.gpsimd.dma_start(res["resid"][i][:],
                                io["resid"][i * 128:(i + 1) * 128, :])
        nc.gpsimd.partition_broadcast(res["lng_b"][:], lng_r[:])
        nc.gpsimd.partition_broadcast(res["lnb_b"][:], lnb_r[:])
        make_identity(nc, res["ident"][:])
        nc.gpsimd.memset(res["vsb"][:], 1.0)

        for rep in range(repeat):
            with tc.tile_pool(name=f"xt{rep}", bufs=3) as xtp, \
                 tc.tile_pool(name=f"ps1{rep}", bufs=2, space="PSUM") as ps1, \
                 tc.tile_pool(name=f"psv{rep}", bufs=2, space="PSUM") as psv:

                pools = {"res": res, "xtp": xtp, "ps1": ps1, "psv": psv}

                # p^T projection (uses ps1 banks, before attention pools open)
                for rt in range(4):
                    ptile = xtp.tile([128, 8 * 512], F8, tag="xt", name="xt")
                    nc.scalar.dma_start(
                        ptile[:],
                        _ap(io["pt"], rt * 512, [[T, 128], [T * 128, 8], [1, 512]]))
                    pp = ps1.tile([128, 512], F32, tag="mm", name="mm")
                    pt3 = ptile[:].rearrange("p (kd n) -> p kd n", kd=8)
                    wp3 = res["wp"][:].rearrange("p (kd n) -> p kd n", kd=8)
                    for k2 in range(4):
                        nc.tensor.matmul(
                            pp[:], lhsT=wp3[:, 2 * k2:2 * k2 + 2, :],
                            rhs=pt3[:, 2 * k2:2 * k2 + 2, :],
                            start=(k2 == 0), stop=(k2 == 3), perf_mode=DR)
                    nc.scalar.activation(res["pt"][:, rt * 512:(rt + 1) * 512],
                                         pp[:], AF.Identity,
                                         scale=1.0 / WSC)

                with tc.tile_pool(name=f"rst{rep}", bufs=4) as rsp, \
                     tc.tile_pool(name=f"psr0{rep}", bufs=2,
                                  space="PSUM") as psr0:
                    pools["rsp"] = rsp
                    pools["psr0"] = psr0
                    _proj_b(nc, tc, io, pools, 0)
                    _proj_b(nc, tc, io, pools, 1)
                    _r_chunks0(nc, tc, io, pools, range(0, 4))
                    _proj_b(nc, tc, io, pools, 2)
                    _r_chunks0(nc, tc, io, pools, range(4, 6))
                    _proj_b(nc, tc, io, pools, 3)
                    _r_chunks0(nc, tc, io, pools, range(6, 8))

            with tc.tile_pool(name=f"rst{rep}b", bufs=4) as rsp, \
                 tc.tile_pool(name=f"skew{rep}", bufs=14) as skp, \
                 tc.tile_pool(name=f"attn{rep}", bufs=5) as atp, \
                 tc.tile_pool(name=f"nrm{rep}", bufs=2) as nrm, \
                 tc.tile_pool(name=f"psr{rep}", bufs=1, space="PSUM") as psr, \
                 tc.tile_pool(name=f"ps2{rep}", bufs=2, space="PSUM") as ps2, \
                 tc.tile_pool(name=f"psav{rep}", bufs=2, space="PSUM") as psav:
                pools = {"res": res, "rsp": rsp, "skp": skp,
                         "atp": atp, "nrm": nrm, "psr": psr, "ps2": ps2,
                         "psav": psav}
                for b in range(B):
                    nxt = _r_chunks(nc, tc, io, pools, b + 1) if b < 3 else []
                    half = len(nxt) // 2
                    _attn_block(nc, tc, io, pools, b, 0, nxt[:half])
                    _attn_block(nc, tc, io, pools, b, 1, nxt[half:])

            # ---- A2A, out-projection, residual, LayerNorm ----
            for h in range(2):
                if io.get("no_cc"):
                    nc.sync.dma_start(io["a2a_out_h"][h][:],
                                      io["a2a_in_h"][h][:])
                else:
                    nc.gpsimd.collective_compute(
                        "AllToAll", ALU.bypass,
                        replica_groups=[list(range(NC))],
                        ins=[io["a2a_in_h"][h][:]],
                        outs=[io["a2a_out_h"][h][:]],
                    )
            for h in range(2):
                nc.sync.dma_start(
                    res["awvf"][h * 64:(h + 1) * 64, :],
                    _ap(io["a2a_out_h"][h], 0,
                        [[512, 64], [32768, 8], [1, 512]]))

            with tc.tile_pool(name=f"outp{rep}", bufs=2) as op_, \
                 tc.tile_pool(name=f"stat{rep}", bufs=2) as stp, \
                 tc.tile_pool(name=f"ps3{rep}", bufs=2, space="PSUM") as ps3:
                awvf, wout = res["awvf"], res["wout"]
                awv3 = awvf[:].rearrange("p (kd n) -> p kd n", kd=8)
                wo3 = wout[:].rearrange("p (kd n) -> p kd n", kd=8)
                for tt in range(4):
                    resid = res["resid"][tt]
                    o = op_.tile([128, D], F32, tag="o", name="o")
                    for n2 in range(2):
                        po = ps3.tile([128, 512], F32, tag="mm", name="mm")
                        for k2 in range(4):
                            nc.tensor.matmul(
                                po[:],
                                lhsT=awv3[:, 2 * k2:2 * k2 + 2,
                                          tt * 128:(tt + 1) * 128],
                                rhs=wo3[:, 2 * k2:2 * k2 + 2,
                                        n2 * 512:n2 * 512 + 512],
                                start=(k2 == 0), stop=(k2 == 3), perf_mode=DR)
                        nc.vector.scalar_tensor_tensor(
                            out=o[:, n2 * 512:(n2 + 1) * 512], in0=po[:],
                            scalar=1.0 / WSC,
                            in1=resid[:, n2 * 512:(n2 + 1) * 512],
                            op0=ALU.mult, op1=ALU.add)
                    bst = stp.tile([128, 12], F32, tag="bst", name="bst")
                    nc.vector.bn_stats(bst[:, 0:6], o[:, 0:512])
                    nc.vector.bn_stats(bst[:, 6:12], o[:, 512:1024])
                    mv = stp.tile([128, 2], F32, tag="mv", name="mv")
                    nc.vector.bn_aggr(mv[:], bst[:])
                    cent = op_.tile([128, D], F32, tag="cent", name="cent")
                    nc.vector.tensor_scalar(out=cent[:], in0=o[:],
                                            scalar1=mv[:, 0:1],
                                            scalar2=None, op0=ALU.subtract)
                    veps = stp.tile([128, 1], F32, tag="veps", name="veps")
                    nc.vector.tensor_scalar(out=veps[:], in0=mv[:, 1:2],
                                            scalar1=LN_EPS, scalar2=None,
                                            op0=ALU.add)
                    std = stp.tile([128, 1], F32, tag="std", name="std")
                    nc.scalar.activation(std[:], veps[:], AF.Sqrt)
                    rstd = stp.tile([128, 1], F32, tag="rstd", name="rstd")
                    nc.vector.reciprocal(rstd[:], std[:])
                    y1 = op_.tile([128, D], F32, tag="o", name="o")
                    nc.vector.scalar_tensor_tensor(
                        out=y1[:], in0=cent[:], scalar=rstd[:], in1=res["lng_b"][:],
                        op0=ALU.mult, op1=ALU.mult)
                    yf = op_.tile([128, D], F32, tag="cent", name="cent")
                    nc.vector.tensor_add(yf[:], y1[:], res["lnb_b"][:])
                    nc.sync.dma_start(out_t[tt * 128:(tt + 1) * 128, :], yf[:])


_compiled = None


def _build(no_cc=False, repeat=1):
    nc = bacc.Bacc("TRN2", target_bir_lowering=False, debug=False, num_devices=NC)
    io = {}
    io["xt"] = nc.dram_tensor("xt", [D, B * T], F8, kind="ExternalInput")
    io["pt"] = nc.dram_tensor("pt", [D, T], F8, kind="ExternalInput")
    io["wk"] = nc.dram_tensor("wk", [D, 128], F8, kind="ExternalInput")
    io["wv"] = nc.dram_tensor("wv", [D, 128], F8, kind="ExternalInput")
    io["wq"] = nc.dram_tensor("wq", [D, 128], F8, kind="ExternalInput")
    io["wp"] = nc.dram_tensor("wp", [D, 128], F8, kind="ExternalInput")
    io["wout"] = nc.dram_tensor("wout", [H * d, D], F8, kind="ExternalInput")
    io["ub"] = nc.dram_tensor("ub", [128, 1], F32, kind="ExternalInput").ap()
    io["vb"] = nc.dram_tensor("vb", [128, 1], F32, kind="ExternalInput").ap()
    io["lng"] = nc.dram_tensor("lng", [1, D], F32, kind="ExternalInput").ap()
    io["lnb"] = nc.dram_tensor("lnb", [1, D], F32, kind="ExternalInput").ap()
    io["resid"] = nc.dram_tensor("resid", [512, D], F32, kind="ExternalInput").ap()
    io["out"] = nc.dram_tensor("out", [512, D], F32, kind="ExternalOutput").ap()
    io["w"] = [[nc.dram_tensor(f"w{b}h{h}", [SZH], BF) for h in range(2)]
               for b in range(B)]
    io["a2a_in_h"] = [nc.dram_tensor(f"a2a_in{h}", [NC, 64, 512], F8)
                      for h in range(2)]
    io["a2a_out_h"] = [nc.dram_tensor(f"a2a_out{h}", [NC, 64, 512], F8)
                       for h in range(2)]
    io["no_cc"] = no_cc
    with tile.TileContext(nc) as tc:
        _body(nc, tc, io, repeat=repeat)
    nc.compile()
    return nc


def _shard(inputs):
    x = np.asarray(inputs["input_"], np.float32)
    pos = np.asarray(inputs["pos_embs"], np.float32)
    mem = np.asarray(inputs["memory"], np.float32)
    u = np.asarray(inputs["u"], np.float32).reshape(-1)
    v = np.asarray(inputs["v"], np.float32).reshape(-1)
    W_kv = np.asarray(inputs["W_kv"], np.float32)
    W_q = np.asarray(inputs["W_q"], np.float32)
    W_p = np.asarray(inputs["W_p"], np.float32)
    W_out = np.asarray(inputs["W_out"], np.float32)
    lng = np.asarray(inputs["ln_g"], np.float32).reshape(1, D)
    lnb = np.asarray(inputs["ln_b"], np.float32).reshape(1, D)

    f8 = ml_dtypes.float8_e4m3
    x_mem = np.concatenate([mem, x], axis=0)
    xt = np.ascontiguousarray(
        x_mem.transpose(2, 1, 0).reshape(D, B * T)).astype(f8)
    pt = np.ascontiguousarray(pos.T).astype(f8)
    wout_b = np.ascontiguousarray(W_out * WSC).astype(f8)

    in_maps = []
    for c in range(NC):
        hs = slice(c * 128, (c + 1) * 128)
        b, i0 = c // 2, (c % 2) * 512
        in_maps.append({
            "xt": xt,
            "pt": pt,
            "wk": _wimg(W_kv[:, hs], f8),
            "wv": _wimg(W_kv[:, H * d + c * 128: H * d + (c + 1) * 128], f8),
            "wq": _wimg(W_q[:, hs], f8),
            "wp": _wimg(W_p[:, hs], f8),
            "wout": wout_b,
            "ub": np.ascontiguousarray(u[hs].reshape(128, 1)),
            "vb": np.ascontiguousarray(v[hs].reshape(128, 1)),
            "lng": lng,
            "lnb": lnb,
            "resid": np.ascontiguousarray(x[i0:i0 + 512, b, :]),
        })
    return in_maps


def _wimg(w_slice, f8):
    # SBUF image for the contiguous weight load: partition p holds
    # W rows {kd*128 + p}, cols kd*128+r = w[kd*128+p, r], scaled x16 (fp8)
    img = (np.asarray(w_slice, np.float32) * 16.0).reshape(8, 128, 128)
    return np.ascontiguousarray(img.transpose(1, 0, 2).reshape(128, 1024)).astype(f8)


LAST_RESULTS = None


def kernel(**inputs):
    global _compiled, LAST_RESULTS
    if _compiled is None:
        _compiled = _build()
    nc = _compiled
    in_maps = _shard(inputs)
    res = run_bass_kernel_spmd(nc, in_maps, core_ids=list(range(NC)))
    LAST_RESULTS = res
    out = np.empty((S, B, D), np.float32)
    for c in range(NC):
        b, i0 = c // 2, (c % 2) * 512
        out[i0:i0 + 512, b, :] = res.results[c]["out"]
    return out

